# revision 25
# baseline (speedup 1.0000x reference)
"""Multi-head attention (B=4, S=1024, H=1024, 16 heads) on 8 trn2 cores.

Sharding: 8 shards = (batch b in 0..3) x (head-half hf in 0..1).
Each core computes attention for 8 heads of one batch and a partial
output projection (row-parallel Wo); host sums the two partials per batch.

Per-core pipeline (matmuls in bf16, PSUM fp32, output bf16):
  - inputs land as big slab DMAs (one per tensor) to minimize DMA count
  - QT/KT computed d-major: QT[dq, tok] = Wq_h^T @ x^T  (N moving)
  - V computed token-major with a ones column appended per head
  - logitsT[k, q] per head via lhsT=KT tile (K=64 contraction); the two
    heads of a pair are emitted adjacently with disjoint PE row groups
    (tile_position (0,0)/(64,0) auto-derived) so hardware overlaps them
  - exp on ACT with per-partition bias fused (no max-subtraction needed:
    logits are O(+-9) so fp32 exp is exact enough)
  - attn@V: lhsT=V_aug [tok,65] -> psum [65, q]; row 64 = softmax denom
  - normalize via reciprocal + gpsimd partition broadcast + DVE multiply
  - out_partial = attnT^T @ Wo accumulated over the 4 pairs in PSUM at the
    end; each [128,512] block is copied to bf16 and DMA'd out immediately
"""

import numpy as np
import ml_dtypes

import concourse.bass as bass
import concourse.tile as tile
from concourse import bacc, mybir
from concourse import bass_utils

F32 = mybir.dt.float32
BF16 = mybir.dt.bfloat16
EXP = mybir.ActivationFunctionType.Exp

S = 1024  # sequence length (tokens)
HID = 1024  # model hidden
DQ = 512  # per-core projected dim (8 heads x 64)
NHL = 8  # local heads per core
DH = 64  # head depth
NK = HID // 128  # 8 contraction tiles over hidden
P = 128
N_CORES = 8

_CACHED_NC = None


def build_program(unroll=1):
    nc = bacc.Bacc("TRN2", target_bir_lowering=False, debug=False)
    xt = nc.dram_tensor("xt", [HID, S], BF16, kind="ExternalInput").ap()
    yt = nc.dram_tensor("yt", [HID, S], BF16, kind="ExternalInput").ap()
    wq = nc.dram_tensor("wq", [HID, DQ], BF16, kind="ExternalInput").ap()
    wk = nc.dram_tensor("wk", [HID, DQ], BF16, kind="ExternalInput").ap()
    wv = nc.dram_tensor("wv", [HID, DQ], BF16, kind="ExternalInput").ap()
    wo = nc.dram_tensor("wo", [DQ, HID], BF16, kind="ExternalInput").ap()
    biasd = nc.dram_tensor("biasd", [P, NK], F32, kind="ExternalInput").ap()
    onesd = nc.dram_tensor("onesd", [P, NHL], BF16, kind="ExternalInput").ap()
    out = nc.dram_tensor("out", [S, HID], BF16, kind="ExternalOutput").ap()

    with tile.TileContext(nc) as tc:
        for _ in range(unroll):
            emit_kernel(tc, out, xt, yt, wq, wk, wv, wo, biasd, onesd)
    nc.compile()
    return nc


def emit_kernel(tc, out, xt, yt, wq, wk, wv, wo, biasd, onesd):
    nc = tc.nc
    with (
        tc.tile_pool(name="wpool", bufs=1) as wpool,
        tc.tile_pool(name="qkv", bufs=1) as qkvpool,
        tc.tile_pool(name="atp", bufs=1) as atpool,
        tc.tile_pool(name="xypool", bufs=1) as xypool,
    ):
        # ---- input DMA: slab DMAs (x/y in two halves so compute can start
        # after the first half lands), earliest-needed first.
        # Slab layout: hidden k-tile k lives at columns [k*W:(k+1)*W].
        def load_slab(eng, name, dram, rows, width, split=1):
            ktiles = rows // P
            t = xypool.tile([P, ktiles * width], BF16, tag=name, name=name)
            dst3 = t[:].rearrange("p (k c) -> p k c", c=width)
            src3 = dram.rearrange("(k p) c -> p k c", p=P)
            step = ktiles // split
            for s in range(split):
                eng.dma_start(
                    dst3[:, s * step : (s + 1) * step, :],
                    src3[:, s * step : (s + 1) * step, :],
                )
            return t

        yt_sb = load_slab(nc.sync, "yts", yt, HID, S, split=2)
        wv_sb = load_slab(nc.scalar, "wvs", wv, HID, DQ)
        bias_sb = wpool.tile([P, NK], F32, tag="bias")
        nc.gpsimd.dma_start(bias_sb[:], biasd[:])
        vones_sb = wpool.tile([P, NHL], BF16, tag="vones")
        nc.gpsimd.dma_start(vones_sb[:], onesd[:])
        xt_sb = load_slab(nc.sync, "xts", xt, HID, S, split=2)
        wq_sb = load_slab(nc.scalar, "wqs", wq, HID, DQ)
        wk_sb = load_slab(nc.gpsimd, "wks", wk, HID, DQ)
        wo_sb = load_slab(nc.scalar, "wos", wo, DQ, HID)

        def ksl(slab, k, width, lo, hi):
            return slab[:, k * width + lo : k * width + hi]

        # ---- persistent slabs ----
        qt_sb = [qkvpool.tile([P, S], BF16, tag=f"qt{m}", name=f"qt{m}") for m in range(4)]
        kt_sb = [qkvpool.tile([P, S], BF16, tag=f"kt{m}", name=f"kt{m}") for m in range(4)]
        v_sb = [qkvpool.tile([P, NHL * (DH + 1)], BF16, tag=f"v{m}", name=f"v{m}") for m in range(8)]
        at_sb = [atpool.tile([P, S], BF16, tag=f"at{m}", name=f"at{m}") for m in range(4)]

        with tc.tile_pool(name="expp", bufs=6) as exppool, tc.tile_pool(
            name="smallp", bufs=4
        ) as smallpool:
            def emit_v_group(pool, m):
                # V projection for token tile m, with ones column appended
                ps = pool.tile([P, DQ], F32, tag="ps", name="ps")
                for k in range(NK):
                    nc.tensor.matmul(
                        ps[:],
                        ksl(yt_sb, k, S, m * P, (m + 1) * P),
                        ksl(wv_sb, k, DQ, 0, DQ),
                        start=(k == 0),
                        stop=(k == NK - 1),
                    )
                dst3 = v_sb[m][:].rearrange("p (h c) -> p h c", c=DH + 1)
                src3 = ps[:].rearrange("p (h c) -> p h c", c=DH)
                nc.vector.tensor_copy(dst3[:, :, 0:DH], src3[:, :, :])
                nc.vector.tensor_copy(
                    dst3[:, :, DH : DH + 1],
                    vones_sb[:].rearrange("p (a b) -> p a b", b=1),
                )

            def emit_proj_group(pool, pair, which, n):
                # one n-half of Q (which=0) or K (which=1) for `pair`
                w_sb, src_sb, dst = (
                    (wq_sb, xt_sb, qt_sb),
                    (wk_sb, yt_sb, kt_sb),
                )[which]
                ps = pool.tile([P, 512], F32, tag="ps", name="ps")
                for k in range(NK):
                    nc.tensor.matmul(
                        ps[:],
                        ksl(w_sb, k, DQ, pair * P, (pair + 1) * P),
                        ksl(src_sb, k, S, n * 512, (n + 1) * 512),
                        start=(k == 0),
                        stop=(k == NK - 1),
                    )
                nc.vector.tensor_copy(dst[pair][:, n * 512 : (n + 1) * 512], ps[:])

            # ---- prologue: V projection (k-outer in two m-groups so the
            # first matmuls only wait on the first yt half) + QT/KT pair 0
            pp_pro = tc.alloc_tile_pool(name="pp_pro", bufs=8, space="PSUM")
            for g in range(2):
                ps = [
                    pp_pro.tile([P, DQ], F32, tag="ps", name="ps") for _ in range(4)
                ]
                for k in range(NK):
                    for mi in range(4):
                        m = g * 4 + mi
                        nc.tensor.matmul(
                            ps[mi][:],
                            ksl(yt_sb, k, S, m * P, (m + 1) * P),
                            ksl(wv_sb, k, DQ, 0, DQ),
                            start=(k == 0),
                            stop=(k == NK - 1),
                        )
                for mi in range(4):
                    m = g * 4 + mi
                    dst3 = v_sb[m][:].rearrange("p (h c) -> p h c", c=DH + 1)
                    src3 = ps[mi][:].rearrange("p (h c) -> p h c", c=DH)
                    nc.vector.tensor_copy(dst3[:, :, 0:DH], src3[:, :, :])
                    nc.vector.tensor_copy(
                        dst3[:, :, DH : DH + 1],
                        vones_sb[:].rearrange("p (a b) -> p a b", b=1),
                    )
            for which in range(2):
                for n in range(2):
                    emit_proj_group(pp_pro, 0, which, n)
            pp_pro.release()

            # ---- attention phase pools: lg 3 + av 2x2 + pj 1 = 8 banks ----
            pp_lg = tc.alloc_tile_pool(name="pp_lg", bufs=3, space="PSUM")
            pp_av = tc.alloc_tile_pool(name="pp_av", bufs=1, space="PSUM")
            pp_pj = tc.alloc_tile_pool(name="pp_pj", bufs=1, space="PSUM")

            def emit_pair_attention(pair, filler=None):
                # both heads of the pair advance together so their logits
                # matmuls (K=64, disjoint PE row groups) sit adjacent in the
                # PE queue and overlap on hardware. `filler` emits extra PE
                # work (V / next-pair projections) inside the ACT-bound loop.
                av = [
                    pp_av.tile([DH + 1, S], F32, tag=f"av{hi}", name=f"av{hi}")
                    for hi in range(2)
                ]
                for sk in range(NK):
                    if filler is not None:
                        filler(sk)
                    e = [None, None]
                    for n in range(2):
                        lg = [None, None]
                        for hi in range(2):
                            base = hi * DH
                            lg[hi] = pp_lg.tile([P, 512], F32, tag="lg", name="lg")
                            nc.tensor.matmul(
                                lg[hi][:],
                                kt_sb[pair][base : base + DH, sk * P : (sk + 1) * P],
                                qt_sb[pair][base : base + DH, n * 512 : (n + 1) * 512],
                                start=True,
                                stop=True,
                            )
                        for hi in range(2):
                            if n == 0:
                                e[hi] = exppool.tile([P, S], BF16, tag="exp", name="exp")
                            nc.scalar.activation(
                                e[hi][:, n * 512 : (n + 1) * 512],
                                lg[hi][:],
                                EXP,
                                bias=bias_sb[:, sk : sk + 1],
                            )
                    for hi in range(2):
                        h = 2 * pair + hi
                        for n in range(2):
                            nc.tensor.matmul(
                                av[hi][:, n * 512 : (n + 1) * 512],
                                v_sb[sk][:, h * (DH + 1) : (h + 1) * (DH + 1)],
                                e[hi][:, n * 512 : (n + 1) * 512],
                                start=(sk == 0),
                                stop=(sk == NK - 1),
                            )
                # Unload the PSUM accumulators to SBUF on ACT (idle at the
                # pair boundary) so the PSUM banks free up immediately for
                # the next pair. The rest of the normalize runs later, off
                # the critical path.
                avs = []
                for hi in range(2):
                    t = smallpool.tile([DH + 1, S], F32, tag="avs", name="avs")
                    if hi == 0:
                        nc.scalar.copy(t[:], av[hi][:])
                    else:
                        nc.vector.tensor_copy(t[:], av[hi][:])
                    avs.append(t)
                return avs

            def emit_pair_normalize(pair, avs):
                # recips on DVE (after any pending projection copies); the
                # broadcasts go to gpsimd, and the two multiplies are split
                # across gpsimd and DVE so the chain finishes sooner
                rc, bc = [], []
                for hi in range(2):
                    rc.append(smallpool.tile([1, S], F32, tag="rc", name="rc"))
                    nc.vector.reciprocal(rc[hi][:], avs[hi][DH : DH + 1, :])
                for hi in range(2):
                    bc.append(smallpool.tile([DH, S], F32, tag="bcsb", name="bcsb"))
                    nc.gpsimd.partition_broadcast(bc[hi][:], rc[hi][:])
                for hi in range(2):
                    base = hi * DH
                    eng = nc.gpsimd if hi == 0 else nc.vector
                    eng.tensor_mul(
                        at_sb[pair][base : base + DH, :], avs[hi][0:DH, :], bc[hi][:]
                    )

            for pair in range(4):
                avs = emit_pair_attention(pair)
                if pair < 3:
                    for which in range(2):
                        for n in range(2):
                            emit_proj_group(pp_pj, pair + 1, which, n)
                emit_pair_normalize(pair, avs)
            pp_pj.release()
            pp_av.release()
            pp_lg.release()

            # ---- output projection: accumulate the 4 pairs in PSUM, then
            # copy to bf16 and DMA out immediately (streams during the phase)
            pp_wo = tc.alloc_tile_pool(name="pp_wo", bufs=8, space="PSUM")
            for m in range(8):
                for n in range(2):
                    ps = pp_wo.tile([P, 512], F32, tag="wops", name="wops")
                    for pair in range(4):
                        nc.tensor.matmul(
                            ps[:],
                            at_sb[pair][:, m * P : (m + 1) * P],
                            wo_sb[:, pair * 1024 + n * 512 : pair * 1024 + (n + 1) * 512],
                            start=(pair == 0),
                            stop=(pair == 3),
                        )
                    ob = smallpool.tile([P, 512], BF16, tag="ob", name="ob")
                    if (m * 2 + n) % 2 == 0:
                        nc.vector.tensor_copy(ob[:], ps[:])
                    else:
                        nc.scalar.copy(ob[:], ps[:])
                    eng = (nc.sync, nc.gpsimd)[(m * 2 + n) % 2]
                    eng.dma_start(
                        out[m * P : (m + 1) * P, n * 512 : (n + 1) * 512], ob[:]
                    )
            pp_wo.release()


def _prep_in_maps(x, y, bias, Wq, Wk, Wv, Wo):
    x = np.asarray(x, dtype=np.float32)
    y = np.asarray(y, dtype=np.float32)
    bias = np.asarray(bias, dtype=np.float32)
    Wq = np.asarray(Wq, dtype=np.float32)
    Wk = np.asarray(Wk, dtype=np.float32)
    Wv = np.asarray(Wv, dtype=np.float32)
    Wo = np.asarray(Wo, dtype=np.float32)
    scale = 1.0 / np.sqrt(DH)
    dt = ml_dtypes.bfloat16
    in_maps = []
    for c in range(N_CORES):
        b, hf = divmod(c, 2)
        cols = slice(hf * DQ, (hf + 1) * DQ)
        in_maps.append(
            {
                "xt": np.ascontiguousarray(x[b].T).astype(dt),
                "yt": np.ascontiguousarray(y[b].T).astype(dt),
                "wq": np.ascontiguousarray(Wq[:, cols] * scale).astype(dt),
                "wk": np.ascontiguousarray(Wk[:, cols]).astype(dt),
                "wv": np.ascontiguousarray(Wv[:, cols]).astype(dt),
                "wo": np.ascontiguousarray(Wo[cols, :]).astype(dt),
                "biasd": np.ascontiguousarray(bias[b, 0, 0].reshape(NK, P).T),
                "onesd": np.ones((P, NHL), dtype=dt),
            }
        )
    return in_maps


def get_program():
    global _CACHED_NC
    if _CACHED_NC is None:
        _CACHED_NC = build_program()
    return _CACHED_NC


def kernel(x, y, bias, Wq, Wk, Wv, Wo):
    nc = get_program()
    in_maps = _prep_in_maps(x, y, bias, Wq, Wk, Wv, Wo)
    res = bass_utils.run_bass_kernel_spmd(nc, in_maps, core_ids=list(range(N_CORES)))
    B = 4
    out = np.empty((B, S, HID), dtype=np.float32)
    for b in range(B):
        out[b] = res.results[2 * b]["out"].astype(np.float32) + res.results[
            2 * b + 1
        ]["out"].astype(np.float32)
    return out


# revision 36
# speedup vs baseline: 2.1579x; 2.1579x over previous
"""Multi-head attention (B=4, S=1024, H=1024, 16 heads) on 8 trn2 cores.

Sharding: 8 shards = (batch b in 0..3) x (head-half hf in 0..1).
Each core computes attention for 8 heads of one batch and a partial
output projection (row-parallel Wo); host sums the two partials per batch.

Per-core pipeline (matmuls in bf16, PSUM fp32, output bf16):
  - inputs land as big slab DMAs (one per tensor) to minimize DMA count
  - QT/KT computed d-major: QT[dq, tok] = Wq_h^T @ x^T  (N moving)
  - V computed token-major with a ones column appended per head
  - logitsT[k, q] per head via lhsT=KT tile (K=64 contraction); the two
    heads of a pair are emitted adjacently with disjoint PE row groups
    (tile_position (0,0)/(64,0) auto-derived) so hardware overlaps them
  - exp on ACT with per-partition bias fused (no max-subtraction needed:
    logits are O(+-9) so fp32 exp is exact enough)
  - attn@V: lhsT=V_aug [tok,65] -> psum [65, q]; row 64 = softmax denom
  - normalize via reciprocal + gpsimd partition broadcast + DVE multiply
  - out_partial = attnT^T @ Wo accumulated over the 4 pairs in PSUM at the
    end; each [128,512] block is copied to bf16 and DMA'd out immediately
"""

import numpy as np
import ml_dtypes

import concourse.bass as bass
import concourse.tile as tile
from concourse import bacc, mybir
from concourse import bass_utils

F32 = mybir.dt.float32
BF16 = mybir.dt.bfloat16
EXP = mybir.ActivationFunctionType.Exp

S = 1024  # sequence length (tokens)
HID = 1024  # model hidden
DQ = 512  # per-core projected dim (8 heads x 64)
NHL = 8  # local heads per core
DH = 64  # head depth
NK = HID // 128  # 8 contraction tiles over hidden
P = 128
N_CORES = 8

_CACHED_NC = None


def build_program(unroll=1):
    nc = bacc.Bacc("TRN2", target_bir_lowering=False, debug=False)
    xt = nc.dram_tensor("xt", [HID, S], BF16, kind="ExternalInput").ap()
    yt = nc.dram_tensor("yt", [HID, S], BF16, kind="ExternalInput").ap()
    wq = nc.dram_tensor("wq", [HID, DQ], BF16, kind="ExternalInput").ap()
    wk = nc.dram_tensor("wk", [HID, DQ], BF16, kind="ExternalInput").ap()
    wv = nc.dram_tensor("wv", [HID, DQ], BF16, kind="ExternalInput").ap()
    wo = nc.dram_tensor("wo", [DQ, HID], BF16, kind="ExternalInput").ap()
    biasd = nc.dram_tensor("biasd", [P, NK], F32, kind="ExternalInput").ap()
    onesd = nc.dram_tensor("onesd", [P, NHL], BF16, kind="ExternalInput").ap()
    out = nc.dram_tensor("out", [S, HID], BF16, kind="ExternalOutput").ap()

    with tile.TileContext(nc) as tc:
        for _ in range(unroll):
            emit_kernel(tc, out, xt, yt, wq, wk, wv, wo, biasd, onesd)
    nc.compile()
    return nc


def emit_kernel(tc, out, xt, yt, wq, wk, wv, wo, biasd, onesd):
    nc = tc.nc
    with (
        tc.tile_pool(name="wpool", bufs=1) as wpool,
        tc.tile_pool(name="qkv", bufs=1) as qkvpool,
        tc.tile_pool(name="atp", bufs=1) as atpool,
        tc.tile_pool(name="xypool", bufs=1) as xypool,
    ):
        # ---- input DMA: slab DMAs (x/y in two halves so compute can start
        # after the first half lands), earliest-needed first.
        # Slab layout: hidden k-tile k lives at columns [k*W:(k+1)*W].
        def load_slab(eng, name, dram, rows, width, split=1):
            ktiles = rows // P
            t = xypool.tile([P, ktiles * width], BF16, tag=name, name=name)
            dst3 = t[:].rearrange("p (k c) -> p k c", c=width)
            src3 = dram.rearrange("(k p) c -> p k c", p=P)
            step = ktiles // split
            for s in range(split):
                eng.dma_start(
                    dst3[:, s * step : (s + 1) * step, :],
                    src3[:, s * step : (s + 1) * step, :],
                )
            return t

        yt_sb = load_slab(nc.sync, "yts", yt, HID, S, split=2)
        wv_sb = load_slab(nc.scalar, "wvs", wv, HID, DQ)
        bias_sb = wpool.tile([P, NK], F32, tag="bias")
        nc.gpsimd.dma_start(bias_sb[:], biasd[:])
        vones_sb = wpool.tile([P, NHL], BF16, tag="vones")
        nc.gpsimd.dma_start(vones_sb[:], onesd[:])
        xt_sb = load_slab(nc.sync, "xts", xt, HID, S, split=2)
        wq_sb = load_slab(nc.scalar, "wqs", wq, HID, DQ)
        wk_sb = load_slab(nc.gpsimd, "wks", wk, HID, DQ)
        wo_sb = load_slab(nc.scalar, "wos", wo, DQ, HID)

        def ksl(slab, k, width, lo, hi):
            return slab[:, k * width + lo : k * width + hi]

        # ---- persistent slabs ----
        qt_sb = [qkvpool.tile([P, S], BF16, tag=f"qt{m}", name=f"qt{m}") for m in range(4)]
        kt_sb = [qkvpool.tile([P, S], BF16, tag=f"kt{m}", name=f"kt{m}") for m in range(4)]
        v_sb = [qkvpool.tile([P, NHL * (DH + 1)], BF16, tag=f"v{m}", name=f"v{m}") for m in range(8)]
        at_sb = [atpool.tile([P, S], BF16, tag=f"at{m}", name=f"at{m}") for m in range(4)]

        with tc.tile_pool(name="expp", bufs=6) as exppool, tc.tile_pool(
            name="smallp", bufs=4
        ) as smallpool:
            def emit_v_group(pool, m):
                # V projection for token tile m, with ones column appended
                ps = pool.tile([P, DQ], F32, tag="ps", name="ps")
                for k in range(NK):
                    nc.tensor.matmul(
                        ps[:],
                        ksl(yt_sb, k, S, m * P, (m + 1) * P),
                        ksl(wv_sb, k, DQ, 0, DQ),
                        start=(k == 0),
                        stop=(k == NK - 1),
                    )
                dst3 = v_sb[m][:].rearrange("p (h c) -> p h c", c=DH + 1)
                src3 = ps[:].rearrange("p (h c) -> p h c", c=DH)
                nc.vector.tensor_copy(dst3[:, :, 0:DH], src3[:, :, :])
                nc.vector.tensor_copy(
                    dst3[:, :, DH : DH + 1],
                    vones_sb[:].rearrange("p (a b) -> p a b", b=1),
                )

            def emit_proj_group(pool, pair, which, n):
                # one n-half of Q (which=0) or K (which=1) for `pair`
                w_sb, src_sb, dst = (
                    (wq_sb, xt_sb, qt_sb),
                    (wk_sb, yt_sb, kt_sb),
                )[which]
                ps = pool.tile([P, 512], F32, tag="ps", name="ps")
                for k in range(NK):
                    nc.tensor.matmul(
                        ps[:],
                        ksl(w_sb, k, DQ, pair * P, (pair + 1) * P),
                        ksl(src_sb, k, S, n * 512, (n + 1) * 512),
                        start=(k == 0),
                        stop=(k == NK - 1),
                    )
                nc.vector.tensor_copy(dst[pair][:, n * 512 : (n + 1) * 512], ps[:])

            # ---- prologue: V projection (k-outer in two m-groups so the
            # first matmuls only wait on the first yt half) + QT/KT pair 0
            pp_pro = tc.alloc_tile_pool(name="pp_pro", bufs=8, space="PSUM")
            for g in range(2):
                ps = [
                    pp_pro.tile([P, DQ], F32, tag="ps", name="ps") for _ in range(4)
                ]
                for k in range(NK):
                    for mi in range(4):
                        m = g * 4 + mi
                        nc.tensor.matmul(
                            ps[mi][:],
                            ksl(yt_sb, k, S, m * P, (m + 1) * P),
                            ksl(wv_sb, k, DQ, 0, DQ),
                            start=(k == 0),
                            stop=(k == NK - 1),
                        )
                for mi in range(4):
                    m = g * 4 + mi
                    dst3 = v_sb[m][:].rearrange("p (h c) -> p h c", c=DH + 1)
                    src3 = ps[mi][:].rearrange("p (h c) -> p h c", c=DH)
                    nc.vector.tensor_copy(dst3[:, :, 0:DH], src3[:, :, :])
                    nc.vector.tensor_copy(
                        dst3[:, :, DH : DH + 1],
                        vones_sb[:].rearrange("p (a b) -> p a b", b=1),
                    )
            for which in range(2):
                for n in range(2):
                    emit_proj_group(pp_pro, 0, which, n)
            pp_pro.release()

            # ---- attention phase pools: lg 3 + av 2x2 + pj 1 = 8 banks ----
            pp_lg = tc.alloc_tile_pool(name="pp_lg", bufs=3, space="PSUM")
            pp_av = tc.alloc_tile_pool(name="pp_av", bufs=1, space="PSUM")
            pp_pj = tc.alloc_tile_pool(name="pp_pj", bufs=1, space="PSUM")

            def emit_pair_attention(pair, last=False):
                # both heads of the pair advance together so their logits
                # matmuls (K=64, disjoint PE row groups) sit adjacent in the
                # PE queue and overlap on hardware. This pair's own KT second
                # half (deferred from the previous pair's window) is emitted
                # at sk 0 — it is first consumed by lg at sk 4.
                av = [
                    pp_av.tile([DH + 1, S], F32, tag=f"av{hi}", name=f"av{hi}")
                    for hi in range(2)
                ]
                for sk in range(NK):
                    if sk == 0 and pair > 0:
                        emit_proj_group(pp_pj, pair, 1, 1)
                    e = [None, None]
                    for n in range(2):
                        lg = [None, None]
                        for hi in range(2):
                            base = hi * DH
                            lg[hi] = pp_lg.tile([P, 512], F32, tag="lg", name="lg")
                            nc.tensor.matmul(
                                lg[hi][:],
                                kt_sb[pair][base : base + DH, sk * P : (sk + 1) * P],
                                qt_sb[pair][base : base + DH, n * 512 : (n + 1) * 512],
                                start=True,
                                stop=True,
                            )
                        for hi in range(2):
                            if n == 0:
                                e[hi] = exppool.tile([P, S], BF16, tag="exp", name="exp")
                            nc.scalar.activation(
                                e[hi][:, n * 512 : (n + 1) * 512],
                                lg[hi][:],
                                EXP,
                                bias=bias_sb[:, sk : sk + 1],
                            )
                    for hi in range(2):
                        h = 2 * pair + hi
                        for n in range(2):
                            nc.tensor.matmul(
                                av[hi][:, n * 512 : (n + 1) * 512],
                                v_sb[sk][:, h * (DH + 1) : (h + 1) * (DH + 1)],
                                e[hi][:, n * 512 : (n + 1) * 512],
                                start=(sk == 0),
                                stop=(sk == NK - 1),
                            )
                # Unload the PSUM accumulators to SBUF on ACT (idle at the
                # pair boundary) so the PSUM banks free up immediately for
                # the next pair. The rest of the normalize runs later, off
                # the critical path.
                avs = []
                for hi in range(2):
                    t = smallpool.tile([DH + 1, S], F32, tag="avs", name="avs")
                    if hi == 0:
                        nc.scalar.copy(t[:], av[hi][:])
                    else:
                        nc.vector.tensor_copy(t[:], av[hi][:])
                    avs.append(t)
                return avs

            def emit_pair_normalize(pair, avs, last=False):
                # recips on DVE (after any pending projection copies); the
                # broadcasts go to gpsimd, and the two multiplies are split
                # across gpsimd and DVE so the chain finishes sooner
                rc, bc = [], []
                for hi in range(2):
                    rc.append(smallpool.tile([1, S], F32, tag="rc", name="rc"))
                    nc.vector.reciprocal(rc[hi][:], avs[hi][DH : DH + 1, :])
                for hi in range(2):
                    bc.append(smallpool.tile([DH, S], F32, tag="bcsb", name="bcsb"))
                    nc.gpsimd.partition_broadcast(bc[hi][:], rc[hi][:])
                for hi in range(2):
                    base = hi * DH
                    # gpsimd cannot read PSUM; the last pair's avs live there
                    eng = nc.vector if (last or hi == 1) else nc.gpsimd
                    eng.tensor_mul(
                        at_sb[pair][base : base + DH, :], avs[hi][0:DH, :], bc[hi][:]
                    )

            for pair in range(4):
                avs = emit_pair_attention(pair, last=(pair == 3))
                if pair < 3:
                    # next pair's Q and KT first half; its KT second half is
                    # deferred into the next pair's window as PE filler
                    emit_proj_group(pp_pj, pair + 1, 0, 0)
                    emit_proj_group(pp_pj, pair + 1, 0, 1)
                    emit_proj_group(pp_pj, pair + 1, 1, 0)
                emit_pair_normalize(pair, avs, last=(pair == 3))
            pp_pj.release()
            pp_av.release()
            pp_lg.release()

            # ---- output projection: accumulate the 4 pairs in PSUM, then
            # copy to bf16 and DMA out immediately (streams during the phase)
            pp_wo = tc.alloc_tile_pool(name="pp_wo", bufs=8, space="PSUM")
            for m in range(8):
                for n in range(2):
                    ps = pp_wo.tile([P, 512], F32, tag="wops", name="wops")
                    for pair in range(4):
                        nc.tensor.matmul(
                            ps[:],
                            at_sb[pair][:, m * P : (m + 1) * P],
                            wo_sb[:, pair * 1024 + n * 512 : pair * 1024 + (n + 1) * 512],
                            start=(pair == 0),
                            stop=(pair == 3),
                        )
                    ob = smallpool.tile([P, 512], BF16, tag="ob", name="ob")
                    if (m * 2 + n) % 2 == 0:
                        nc.vector.tensor_copy(ob[:], ps[:])
                    else:
                        nc.scalar.copy(ob[:], ps[:])
                    eng = (nc.sync, nc.gpsimd)[(m * 2 + n) % 2]
                    eng.dma_start(
                        out[m * P : (m + 1) * P, n * 512 : (n + 1) * 512], ob[:]
                    )
            pp_wo.release()


def _prep_in_maps(x, y, bias, Wq, Wk, Wv, Wo):
    x = np.asarray(x, dtype=np.float32)
    y = np.asarray(y, dtype=np.float32)
    bias = np.asarray(bias, dtype=np.float32)
    Wq = np.asarray(Wq, dtype=np.float32)
    Wk = np.asarray(Wk, dtype=np.float32)
    Wv = np.asarray(Wv, dtype=np.float32)
    Wo = np.asarray(Wo, dtype=np.float32)
    scale = 1.0 / np.sqrt(DH)
    dt = ml_dtypes.bfloat16
    in_maps = []
    for c in range(N_CORES):
        b, hf = divmod(c, 2)
        cols = slice(hf * DQ, (hf + 1) * DQ)
        in_maps.append(
            {
                "xt": np.ascontiguousarray(x[b].T).astype(dt),
                "yt": np.ascontiguousarray(y[b].T).astype(dt),
                "wq": np.ascontiguousarray(Wq[:, cols] * scale).astype(dt),
                "wk": np.ascontiguousarray(Wk[:, cols]).astype(dt),
                "wv": np.ascontiguousarray(Wv[:, cols]).astype(dt),
                "wo": np.ascontiguousarray(Wo[cols, :]).astype(dt),
                "biasd": np.ascontiguousarray(bias[b, 0, 0].reshape(NK, P).T),
                "onesd": np.ones((P, NHL), dtype=dt),
            }
        )
    return in_maps


def get_program():
    global _CACHED_NC
    if _CACHED_NC is None:
        _CACHED_NC = build_program()
    return _CACHED_NC


def kernel(x, y, bias, Wq, Wk, Wv, Wo):
    nc = get_program()
    in_maps = _prep_in_maps(x, y, bias, Wq, Wk, Wv, Wo)
    res = bass_utils.run_bass_kernel_spmd(nc, in_maps, core_ids=list(range(N_CORES)))
    B = 4
    out = np.empty((B, S, HID), dtype=np.float32)
    for b in range(B):
        out[b] = res.results[2 * b]["out"].astype(np.float32) + res.results[
            2 * b + 1
        ]["out"].astype(np.float32)
    return out


# revision 38
# speedup vs baseline: 2.1652x; 1.0034x over previous
"""Multi-head attention (B=4, S=1024, H=1024, 16 heads) on 8 trn2 cores.

Sharding: 8 shards = (batch b in 0..3) x (head-half hf in 0..1).
Each core computes attention for 8 heads of one batch and a partial
output projection (row-parallel Wo); host sums the two partials per batch.

Per-core pipeline (matmuls in bf16, PSUM fp32, output bf16):
  - inputs land as big slab DMAs (x/y split in two halves) to minimize
    DMA count while letting compute start early
  - QT/KT computed d-major: QT[dq, tok] = Wq_h^T @ x^T  (N moving)
  - V computed token-major (k-outer over 8 PSUM banks in the prologue)
    with a ones column appended per head
  - logitsT[k, q] per head via lhsT=KT tile (K=64 contraction); the two
    heads of a pair are emitted adjacently with disjoint PE row groups
    (tile_position (0,0)/(64,0) auto-derived) so hardware overlaps them
  - exp on ACT with per-partition bias fused (no max-subtraction needed:
    logits are O(+-9) so fp32 exp is exact enough)
  - attn@V: lhsT=V_aug [tok,65] -> psum [65, q]; row 64 = softmax denom
  - av PSUM unloaded to SBUF right away (ACT+DVE in parallel) so the
    banks recycle; normalize (recip -> partition broadcast -> multiply)
    runs off the critical path on DVE/gpsimd
  - out_partial = attnT^T @ Wo accumulated over the 4 pairs in PSUM at the
    end; each [128,512] block is copied to bf16 and DMA'd out immediately
PSUM budget during attention: logits 3 + attn-accum 4 + projections 1 = 8.
"""

import numpy as np
import ml_dtypes

import concourse.bass as bass
import concourse.tile as tile
from concourse import bacc, mybir
from concourse import bass_utils

F32 = mybir.dt.float32
BF16 = mybir.dt.bfloat16
EXP = mybir.ActivationFunctionType.Exp

S = 1024  # sequence length (tokens)
HID = 1024  # model hidden
DQ = 512  # per-core projected dim (8 heads x 64)
NHL = 8  # local heads per core
DH = 64  # head depth
NK = HID // 128  # 8 contraction tiles over hidden
P = 128
N_CORES = 8

_CACHED_NC = None


def build_program(unroll=1):
    nc = bacc.Bacc("TRN2", target_bir_lowering=False, debug=False)
    xt = nc.dram_tensor("xt", [HID, S], BF16, kind="ExternalInput").ap()
    yt = nc.dram_tensor("yt", [HID, S], BF16, kind="ExternalInput").ap()
    wq = nc.dram_tensor("wq", [HID, DQ], BF16, kind="ExternalInput").ap()
    wk = nc.dram_tensor("wk", [HID, DQ], BF16, kind="ExternalInput").ap()
    wv = nc.dram_tensor("wv", [HID, DQ], BF16, kind="ExternalInput").ap()
    wo = nc.dram_tensor("wo", [DQ, HID], BF16, kind="ExternalInput").ap()
    biasd = nc.dram_tensor("biasd", [P, NK], F32, kind="ExternalInput").ap()
    onesd = nc.dram_tensor("onesd", [P, NHL], BF16, kind="ExternalInput").ap()
    out = nc.dram_tensor("out", [S, HID], BF16, kind="ExternalOutput").ap()

    with tile.TileContext(nc) as tc:
        for _ in range(unroll):
            emit_kernel(tc, out, xt, yt, wq, wk, wv, wo, biasd, onesd)
    nc.compile()
    return nc


def emit_kernel(tc, out, xt, yt, wq, wk, wv, wo, biasd, onesd):
    nc = tc.nc
    with (
        tc.tile_pool(name="wpool", bufs=1) as wpool,
        tc.tile_pool(name="qkv", bufs=1) as qkvpool,
        tc.tile_pool(name="atp", bufs=1) as atpool,
        tc.tile_pool(name="xypool", bufs=1) as xypool,
    ):
        # ---- input DMA: slab DMAs (x/y in two halves so compute can start
        # after the first half lands), earliest-needed first.
        # Slab layout: hidden k-tile k lives at columns [k*W:(k+1)*W].
        def load_slab(eng, name, dram, rows, width, split=1):
            ktiles = rows // P
            t = xypool.tile([P, ktiles * width], BF16, tag=name, name=name)
            dst3 = t[:].rearrange("p (k c) -> p k c", c=width)
            src3 = dram.rearrange("(k p) c -> p k c", p=P)
            step = ktiles // split
            for s in range(split):
                eng.dma_start(
                    dst3[:, s * step : (s + 1) * step, :],
                    src3[:, s * step : (s + 1) * step, :],
                )
            return t

        yt_sb = load_slab(nc.sync, "yts", yt, HID, S, split=2)
        wv_sb = load_slab(nc.scalar, "wvs", wv, HID, DQ)
        bias_sb = wpool.tile([P, NK], F32, tag="bias")
        nc.gpsimd.dma_start(bias_sb[:], biasd[:])
        vones_sb = wpool.tile([P, NHL], BF16, tag="vones")
        nc.gpsimd.dma_start(vones_sb[:], onesd[:])
        xt_sb = load_slab(nc.sync, "xts", xt, HID, S, split=2)
        wq_sb = load_slab(nc.scalar, "wqs", wq, HID, DQ)
        wk_sb = load_slab(nc.gpsimd, "wks", wk, HID, DQ)
        wo_sb = load_slab(nc.scalar, "wos", wo, DQ, HID)

        def ksl(slab, k, width, lo, hi):
            return slab[:, k * width + lo : k * width + hi]

        # ---- persistent slabs ----
        qt_sb = [qkvpool.tile([P, S], BF16, tag=f"qt{m}", name=f"qt{m}") for m in range(4)]
        kt_sb = [qkvpool.tile([P, S], BF16, tag=f"kt{m}", name=f"kt{m}") for m in range(4)]
        v_sb = [qkvpool.tile([P, NHL * (DH + 1)], BF16, tag=f"v{m}", name=f"v{m}") for m in range(8)]
        at_sb = [atpool.tile([P, S], BF16, tag=f"at{m}", name=f"at{m}") for m in range(4)]

        with tc.tile_pool(name="expp", bufs=6) as exppool, tc.tile_pool(
            name="smallp", bufs=4
        ) as smallpool:
            def emit_proj_group(pool, pair, which, n):
                # one n-half of Q (which=0) or K (which=1) for `pair`
                w_sb, src_sb, dst = (
                    (wq_sb, xt_sb, qt_sb),
                    (wk_sb, yt_sb, kt_sb),
                )[which]
                ps = pool.tile([P, 512], F32, tag="ps", name="ps")
                for k in range(NK):
                    nc.tensor.matmul(
                        ps[:],
                        ksl(w_sb, k, DQ, pair * P, (pair + 1) * P),
                        ksl(src_sb, k, S, n * 512, (n + 1) * 512),
                        start=(k == 0),
                        stop=(k == NK - 1),
                    )
                nc.vector.tensor_copy(dst[pair][:, n * 512 : (n + 1) * 512], ps[:])

            # ---- prologue: V projection (k-outer in two m-groups so the
            # first matmuls only wait on the first yt half) + QT/KT pair 0
            pp_pro = tc.alloc_tile_pool(name="pp_pro", bufs=8, space="PSUM")
            for g in range(2):
                ps = [
                    pp_pro.tile([P, DQ], F32, tag="ps", name="ps") for _ in range(4)
                ]
                for k in range(NK):
                    for mi in range(4):
                        m = g * 4 + mi
                        nc.tensor.matmul(
                            ps[mi][:],
                            ksl(yt_sb, k, S, m * P, (m + 1) * P),
                            ksl(wv_sb, k, DQ, 0, DQ),
                            start=(k == 0),
                            stop=(k == NK - 1),
                        )
                for mi in range(4):
                    m = g * 4 + mi
                    dst3 = v_sb[m][:].rearrange("p (h c) -> p h c", c=DH + 1)
                    src3 = ps[mi][:].rearrange("p (h c) -> p h c", c=DH)
                    nc.vector.tensor_copy(dst3[:, :, 0:DH], src3[:, :, :])
                    nc.vector.tensor_copy(
                        dst3[:, :, DH : DH + 1],
                        vones_sb[:].rearrange("p (a b) -> p a b", b=1),
                    )
            for which in range(2):
                for n in range(2):
                    emit_proj_group(pp_pro, 0, which, n)
            pp_pro.release()

            # ---- attention phase pools: lg 3 + av 2x2 + pj 1 = 8 banks ----
            pp_lg = tc.alloc_tile_pool(name="pp_lg", bufs=3, space="PSUM")
            pp_av = tc.alloc_tile_pool(name="pp_av", bufs=1, space="PSUM")
            pp_pj = tc.alloc_tile_pool(name="pp_pj", bufs=1, space="PSUM")

            def emit_pair_attention(pair, last=False):
                # both heads of the pair advance together so their logits
                # matmuls (K=64, disjoint PE row groups) sit adjacent in the
                # PE queue and overlap on hardware. This pair's own KT second
                # half (deferred from the previous pair's window) is emitted
                # at sk 0 — it is first consumed by lg at sk 4.
                av = [
                    pp_av.tile([DH + 1, S], F32, tag=f"av{hi}", name=f"av{hi}")
                    for hi in range(2)
                ]
                for sk in range(NK):
                    if sk == 0 and pair > 0:
                        emit_proj_group(pp_pj, pair, 1, 1)
                    e = [None, None]
                    for n in range(2):
                        lg = [None, None]
                        for hi in range(2):
                            base = hi * DH
                            lg[hi] = pp_lg.tile([P, 512], F32, tag="lg", name="lg")
                            nc.tensor.matmul(
                                lg[hi][:],
                                kt_sb[pair][base : base + DH, sk * P : (sk + 1) * P],
                                qt_sb[pair][base : base + DH, n * 512 : (n + 1) * 512],
                                start=True,
                                stop=True,
                            )
                        for hi in range(2):
                            if n == 0:
                                e[hi] = exppool.tile([P, S], BF16, tag="exp", name="exp")
                            nc.scalar.activation(
                                e[hi][:, n * 512 : (n + 1) * 512],
                                lg[hi][:],
                                EXP,
                                bias=bias_sb[:, sk : sk + 1],
                            )
                    for hi in range(2):
                        h = 2 * pair + hi
                        for n in range(2):
                            nc.tensor.matmul(
                                av[hi][:, n * 512 : (n + 1) * 512],
                                v_sb[sk][:, h * (DH + 1) : (h + 1) * (DH + 1)],
                                e[hi][:, n * 512 : (n + 1) * 512],
                                start=(sk == 0),
                                stop=(sk == NK - 1),
                            )
                # Unload the PSUM accumulators to SBUF on ACT (idle at the
                # pair boundary) so the PSUM banks free up immediately for
                # the next pair. The rest of the normalize runs later, off
                # the critical path.
                avs = []
                for hi in range(2):
                    t = smallpool.tile([DH + 1, S], F32, tag="avs", name="avs")
                    if hi == 0:
                        nc.scalar.copy(t[:], av[hi][:])
                    else:
                        nc.vector.tensor_copy(t[:], av[hi][:])
                    avs.append(t)
                return avs

            def emit_pair_normalize(pair, avs, last=False):
                # recips on DVE (after any pending projection copies); the
                # broadcasts go to gpsimd, and the two multiplies are split
                # across gpsimd and DVE so the chain finishes sooner
                rc, bc = [], []
                for hi in range(2):
                    rc.append(smallpool.tile([1, S], F32, tag="rc", name="rc"))
                    nc.vector.reciprocal(rc[hi][:], avs[hi][DH : DH + 1, :])
                for hi in range(2):
                    bc.append(smallpool.tile([DH, S], F32, tag="bcsb", name="bcsb"))
                    nc.gpsimd.partition_broadcast(bc[hi][:], rc[hi][:])
                for hi in range(2):
                    base = hi * DH
                    # gpsimd cannot read PSUM; the last pair's avs live there
                    eng = nc.vector if (last or hi == 1) else nc.gpsimd
                    eng.tensor_mul(
                        at_sb[pair][base : base + DH, :], avs[hi][0:DH, :], bc[hi][:]
                    )

            for pair in range(4):
                avs = emit_pair_attention(pair, last=(pair == 3))
                if pair < 3:
                    # next pair's Q and KT first half; its KT second half is
                    # deferred into the next pair's window as PE filler
                    emit_proj_group(pp_pj, pair + 1, 0, 0)
                    emit_proj_group(pp_pj, pair + 1, 0, 1)
                    emit_proj_group(pp_pj, pair + 1, 1, 0)
                emit_pair_normalize(pair, avs, last=(pair == 3))
            pp_pj.release()
            pp_av.release()
            pp_lg.release()

            # ---- output projection: accumulate the 4 pairs in PSUM, then
            # copy to bf16 and DMA out immediately (streams during the phase)
            pp_wo = tc.alloc_tile_pool(name="pp_wo", bufs=8, space="PSUM")
            for m in range(8):
                for n in range(2):
                    ps = pp_wo.tile([P, 512], F32, tag="wops", name="wops")
                    for pair in range(4):
                        nc.tensor.matmul(
                            ps[:],
                            at_sb[pair][:, m * P : (m + 1) * P],
                            wo_sb[:, pair * 1024 + n * 512 : pair * 1024 + (n + 1) * 512],
                            start=(pair == 0),
                            stop=(pair == 3),
                        )
                    ob = smallpool.tile([P, 512], BF16, tag="ob", name="ob")
                    if (m * 2 + n) % 2 == 0:
                        nc.vector.tensor_copy(ob[:], ps[:])
                    else:
                        nc.scalar.copy(ob[:], ps[:])
                    eng = (nc.sync, nc.gpsimd)[(m * 2 + n) % 2]
                    eng.dma_start(
                        out[m * P : (m + 1) * P, n * 512 : (n + 1) * 512], ob[:]
                    )
            pp_wo.release()


def _prep_in_maps(x, y, bias, Wq, Wk, Wv, Wo):
    x = np.asarray(x, dtype=np.float32)
    y = np.asarray(y, dtype=np.float32)
    bias = np.asarray(bias, dtype=np.float32)
    Wq = np.asarray(Wq, dtype=np.float32)
    Wk = np.asarray(Wk, dtype=np.float32)
    Wv = np.asarray(Wv, dtype=np.float32)
    Wo = np.asarray(Wo, dtype=np.float32)
    scale = 1.0 / np.sqrt(DH)
    dt = ml_dtypes.bfloat16
    in_maps = []
    for c in range(N_CORES):
        b, hf = divmod(c, 2)
        cols = slice(hf * DQ, (hf + 1) * DQ)
        in_maps.append(
            {
                "xt": np.ascontiguousarray(x[b].T).astype(dt),
                "yt": np.ascontiguousarray(y[b].T).astype(dt),
                "wq": np.ascontiguousarray(Wq[:, cols] * scale).astype(dt),
                "wk": np.ascontiguousarray(Wk[:, cols]).astype(dt),
                "wv": np.ascontiguousarray(Wv[:, cols]).astype(dt),
                "wo": np.ascontiguousarray(Wo[cols, :]).astype(dt),
                "biasd": np.ascontiguousarray(bias[b, 0, 0].reshape(NK, P).T),
                "onesd": np.ones((P, NHL), dtype=dt),
            }
        )
    return in_maps


def get_program():
    global _CACHED_NC
    if _CACHED_NC is None:
        _CACHED_NC = build_program()
    return _CACHED_NC


def kernel(x, y, bias, Wq, Wk, Wv, Wo):
    nc = get_program()
    in_maps = _prep_in_maps(x, y, bias, Wq, Wk, Wv, Wo)
    res = bass_utils.run_bass_kernel_spmd(nc, in_maps, core_ids=list(range(N_CORES)))
    B = 4
    out = np.empty((B, S, HID), dtype=np.float32)
    for b in range(B):
        out[b] = res.results[2 * b]["out"].astype(np.float32) + res.results[
            2 * b + 1
        ]["out"].astype(np.float32)
    return out


# revision 39
# speedup vs baseline: 2.2382x; 1.0337x over previous
"""Multi-head attention (B=4, S=1024, H=1024, 16 heads) on 8 trn2 cores.

Sharding: 8 shards = (batch b in 0..3) x (head-half hf in 0..1).
Each core computes attention for 8 heads of one batch and a partial
output projection (row-parallel Wo); host sums the two partials per batch.

Per-core pipeline (matmuls in bf16, PSUM fp32, output bf16):
  - inputs land as big slab DMAs (x/y split in two halves) to minimize
    DMA count while letting compute start early
  - QT/KT computed d-major: QT[dq, tok] = Wq_h^T @ x^T  (N moving)
  - V computed token-major (k-outer over 8 PSUM banks in the prologue)
    with a ones column appended per head
  - logitsT[k, q] per head via lhsT=KT tile (K=64 contraction); the two
    heads of a pair are emitted adjacently with disjoint PE row groups
    (tile_position (0,0)/(64,0) auto-derived) so hardware overlaps them
  - exp on ACT with per-partition bias fused (no max-subtraction needed:
    logits are O(+-9) so fp32 exp is exact enough)
  - attn@V: lhsT=V_aug [tok,65] -> psum [65, q]; row 64 = softmax denom
  - av PSUM unloaded to SBUF right away (ACT+DVE in parallel) so the
    banks recycle; normalize (recip -> partition broadcast -> multiply)
    runs off the critical path on DVE/gpsimd
  - out_partial = attnT^T @ Wo accumulated over the 4 pairs in PSUM at the
    end; each [128,512] block is copied to bf16 and DMA'd out immediately
PSUM budget during attention: logits 3 + attn-accum 4 + projections 1 = 8.
"""

import numpy as np
import ml_dtypes

import concourse.bass as bass
import concourse.tile as tile
from concourse import bacc, mybir
from concourse import bass_utils

F32 = mybir.dt.float32
BF16 = mybir.dt.bfloat16
EXP = mybir.ActivationFunctionType.Exp

S = 1024  # sequence length (tokens)
HID = 1024  # model hidden
DQ = 512  # per-core projected dim (8 heads x 64)
NHL = 8  # local heads per core
DH = 64  # head depth
NK = HID // 128  # 8 contraction tiles over hidden
P = 128
N_CORES = 8

_CACHED_NC = None


def build_program(unroll=1):
    nc = bacc.Bacc("TRN2", target_bir_lowering=False, debug=False)
    xt = nc.dram_tensor("xt", [HID, S], BF16, kind="ExternalInput").ap()
    yt = nc.dram_tensor("yt", [HID, S], BF16, kind="ExternalInput").ap()
    wq = nc.dram_tensor("wq", [HID, DQ], BF16, kind="ExternalInput").ap()
    wk = nc.dram_tensor("wk", [HID, DQ], BF16, kind="ExternalInput").ap()
    wv = nc.dram_tensor("wv", [HID, DQ], BF16, kind="ExternalInput").ap()
    wo = nc.dram_tensor("wo", [DQ, HID], BF16, kind="ExternalInput").ap()
    biasd = nc.dram_tensor("biasd", [P, NK], F32, kind="ExternalInput").ap()
    onesd = nc.dram_tensor("onesd", [P, NHL], BF16, kind="ExternalInput").ap()
    out = nc.dram_tensor("out", [S, HID], BF16, kind="ExternalOutput").ap()

    with tile.TileContext(nc) as tc:
        for _ in range(unroll):
            emit_kernel(tc, out, xt, yt, wq, wk, wv, wo, biasd, onesd)
    nc.compile()
    return nc


def emit_kernel(tc, out, xt, yt, wq, wk, wv, wo, biasd, onesd):
    nc = tc.nc
    with (
        tc.tile_pool(name="wpool", bufs=1) as wpool,
        tc.tile_pool(name="qkv", bufs=1) as qkvpool,
        tc.tile_pool(name="atp", bufs=1) as atpool,
        tc.tile_pool(name="xypool", bufs=1) as xypool,
    ):
        # ---- input DMA: slab DMAs (x/y in two halves so compute can start
        # after the first half lands), earliest-needed first.
        # Slab layout: hidden k-tile k lives at columns [k*W:(k+1)*W].
        def load_slab(eng, name, dram, rows, width, split=1):
            ktiles = rows // P
            t = xypool.tile([P, ktiles * width], BF16, tag=name, name=name)
            dst3 = t[:].rearrange("p (k c) -> p k c", c=width)
            src3 = dram.rearrange("(k p) c -> p k c", p=P)
            step = ktiles // split
            for s in range(split):
                eng.dma_start(
                    dst3[:, s * step : (s + 1) * step, :],
                    src3[:, s * step : (s + 1) * step, :],
                )
            return t

        yt_sb = load_slab(nc.sync, "yts", yt, HID, S, split=2)
        wv_sb = load_slab(nc.scalar, "wvs", wv, HID, DQ)
        bias_sb = wpool.tile([P, NK], F32, tag="bias")
        nc.gpsimd.dma_start(bias_sb[:], biasd[:])
        vones_sb = wpool.tile([P, NHL], BF16, tag="vones")
        nc.gpsimd.dma_start(vones_sb[:], onesd[:])
        xt_sb = load_slab(nc.sync, "xts", xt, HID, S, split=2)
        wq_sb = load_slab(nc.scalar, "wqs", wq, HID, DQ)
        wk_sb = load_slab(nc.gpsimd, "wks", wk, HID, DQ)
        wo_sb = load_slab(nc.scalar, "wos", wo, DQ, HID)

        def ksl(slab, k, width, lo, hi):
            return slab[:, k * width + lo : k * width + hi]

        # ---- persistent slabs ----
        qt_sb = [qkvpool.tile([P, S], BF16, tag=f"qt{m}", name=f"qt{m}") for m in range(4)]
        kt_sb = [qkvpool.tile([P, S], BF16, tag=f"kt{m}", name=f"kt{m}") for m in range(4)]
        v_sb = [qkvpool.tile([P, NHL * (DH + 1)], BF16, tag=f"v{m}", name=f"v{m}") for m in range(8)]
        at_sb = [atpool.tile([P, S], BF16, tag=f"at{m}", name=f"at{m}") for m in range(4)]

        with tc.tile_pool(name="expp", bufs=8) as exppool, tc.tile_pool(
            name="smallp", bufs=4
        ) as smallpool:
            def emit_proj_group(pool, pair, which, n):
                # one n-half of Q (which=0) or K (which=1) for `pair`
                w_sb, src_sb, dst = (
                    (wq_sb, xt_sb, qt_sb),
                    (wk_sb, yt_sb, kt_sb),
                )[which]
                ps = pool.tile([P, 512], F32, tag="ps", name="ps")
                for k in range(NK):
                    nc.tensor.matmul(
                        ps[:],
                        ksl(w_sb, k, DQ, pair * P, (pair + 1) * P),
                        ksl(src_sb, k, S, n * 512, (n + 1) * 512),
                        start=(k == 0),
                        stop=(k == NK - 1),
                    )
                nc.vector.tensor_copy(dst[pair][:, n * 512 : (n + 1) * 512], ps[:])

            # ---- prologue: V projection (k-outer in two m-groups so the
            # first matmuls only wait on the first yt half) + QT/KT pair 0
            pp_pro = tc.alloc_tile_pool(name="pp_pro", bufs=8, space="PSUM")
            for g in range(2):
                ps = [
                    pp_pro.tile([P, DQ], F32, tag="ps", name="ps") for _ in range(4)
                ]
                for k in range(NK):
                    for mi in range(4):
                        m = g * 4 + mi
                        nc.tensor.matmul(
                            ps[mi][:],
                            ksl(yt_sb, k, S, m * P, (m + 1) * P),
                            ksl(wv_sb, k, DQ, 0, DQ),
                            start=(k == 0),
                            stop=(k == NK - 1),
                        )
                for mi in range(4):
                    m = g * 4 + mi
                    dst3 = v_sb[m][:].rearrange("p (h c) -> p h c", c=DH + 1)
                    src3 = ps[mi][:].rearrange("p (h c) -> p h c", c=DH)
                    nc.vector.tensor_copy(dst3[:, :, 0:DH], src3[:, :, :])
                    nc.vector.tensor_copy(
                        dst3[:, :, DH : DH + 1],
                        vones_sb[:].rearrange("p (a b) -> p a b", b=1),
                    )
            for which in range(2):
                for n in range(2):
                    emit_proj_group(pp_pro, 0, which, n)
            pp_pro.release()

            # ---- attention phase pools: lg 3 + av 2x2 + pj 1 = 8 banks ----
            pp_lg = tc.alloc_tile_pool(name="pp_lg", bufs=3, space="PSUM")
            pp_av = tc.alloc_tile_pool(name="pp_av", bufs=1, space="PSUM")
            pp_pj = tc.alloc_tile_pool(name="pp_pj", bufs=1, space="PSUM")

            def emit_pair_attention(pair, last=False):
                # both heads of the pair advance together so their logits
                # matmuls (K=64, disjoint PE row groups) sit adjacent in the
                # PE queue and overlap on hardware. This pair's own KT second
                # half (deferred from the previous pair's window) is emitted
                # at sk 0 — it is first consumed by lg at sk 4.
                av = [
                    pp_av.tile([DH + 1, S], F32, tag=f"av{hi}", name=f"av{hi}")
                    for hi in range(2)
                ]
                for sk in range(NK):
                    if sk == 0 and pair > 0:
                        emit_proj_group(pp_pj, pair, 1, 1)
                    e = [None, None]
                    for n in range(2):
                        lg = [None, None]
                        for hi in range(2):
                            base = hi * DH
                            lg[hi] = pp_lg.tile([P, 512], F32, tag="lg", name="lg")
                            nc.tensor.matmul(
                                lg[hi][:],
                                kt_sb[pair][base : base + DH, sk * P : (sk + 1) * P],
                                qt_sb[pair][base : base + DH, n * 512 : (n + 1) * 512],
                                start=True,
                                stop=True,
                            )
                        for hi in range(2):
                            if n == 0:
                                e[hi] = exppool.tile([P, S], BF16, tag="exp", name="exp")
                            nc.scalar.activation(
                                e[hi][:, n * 512 : (n + 1) * 512],
                                lg[hi][:],
                                EXP,
                                bias=bias_sb[:, sk : sk + 1],
                            )
                    for hi in range(2):
                        h = 2 * pair + hi
                        for n in range(2):
                            nc.tensor.matmul(
                                av[hi][:, n * 512 : (n + 1) * 512],
                                v_sb[sk][:, h * (DH + 1) : (h + 1) * (DH + 1)],
                                e[hi][:, n * 512 : (n + 1) * 512],
                                start=(sk == 0),
                                stop=(sk == NK - 1),
                            )
                # Unload the PSUM accumulators to SBUF on ACT (idle at the
                # pair boundary) so the PSUM banks free up immediately for
                # the next pair. The rest of the normalize runs later, off
                # the critical path.
                avs = []
                for hi in range(2):
                    t = smallpool.tile([DH + 1, S], F32, tag="avs", name="avs")
                    if hi == 0:
                        nc.scalar.copy(t[:], av[hi][:])
                    else:
                        nc.vector.tensor_copy(t[:], av[hi][:])
                    avs.append(t)
                return avs

            def emit_pair_normalize(pair, avs, last=False):
                # recips on DVE (after any pending projection copies); the
                # broadcasts go to gpsimd, and the two multiplies are split
                # across gpsimd and DVE so the chain finishes sooner
                rc, bc = [], []
                for hi in range(2):
                    rc.append(smallpool.tile([1, S], F32, tag="rc", name="rc"))
                    nc.vector.reciprocal(rc[hi][:], avs[hi][DH : DH + 1, :])
                for hi in range(2):
                    bc.append(smallpool.tile([DH, S], F32, tag="bcsb", name="bcsb"))
                    nc.gpsimd.partition_broadcast(bc[hi][:], rc[hi][:])
                for hi in range(2):
                    base = hi * DH
                    # gpsimd cannot read PSUM; the last pair's avs live there
                    eng = nc.vector if (last or hi == 1) else nc.gpsimd
                    eng.tensor_mul(
                        at_sb[pair][base : base + DH, :], avs[hi][0:DH, :], bc[hi][:]
                    )

            for pair in range(4):
                avs = emit_pair_attention(pair, last=(pair == 3))
                if pair < 3:
                    # next pair's Q and KT first half; its KT second half is
                    # deferred into the next pair's window as PE filler
                    emit_proj_group(pp_pj, pair + 1, 0, 0)
                    emit_proj_group(pp_pj, pair + 1, 0, 1)
                    emit_proj_group(pp_pj, pair + 1, 1, 0)
                emit_pair_normalize(pair, avs, last=(pair == 3))
            pp_pj.release()
            pp_av.release()
            pp_lg.release()

            # ---- output projection: accumulate the 4 pairs in PSUM, then
            # copy to bf16 and DMA out immediately (streams during the phase)
            pp_wo = tc.alloc_tile_pool(name="pp_wo", bufs=8, space="PSUM")
            for m in range(8):
                for n in range(2):
                    ps = pp_wo.tile([P, 512], F32, tag="wops", name="wops")
                    for pair in range(4):
                        nc.tensor.matmul(
                            ps[:],
                            at_sb[pair][:, m * P : (m + 1) * P],
                            wo_sb[:, pair * 1024 + n * 512 : pair * 1024 + (n + 1) * 512],
                            start=(pair == 0),
                            stop=(pair == 3),
                        )
                    ob = smallpool.tile([P, 512], BF16, tag="ob", name="ob")
                    if (m * 2 + n) % 2 == 0:
                        nc.vector.tensor_copy(ob[:], ps[:])
                    else:
                        nc.scalar.copy(ob[:], ps[:])
                    eng = (nc.sync, nc.gpsimd)[(m * 2 + n) % 2]
                    eng.dma_start(
                        out[m * P : (m + 1) * P, n * 512 : (n + 1) * 512], ob[:]
                    )
            pp_wo.release()


def _prep_in_maps(x, y, bias, Wq, Wk, Wv, Wo):
    x = np.asarray(x, dtype=np.float32)
    y = np.asarray(y, dtype=np.float32)
    bias = np.asarray(bias, dtype=np.float32)
    Wq = np.asarray(Wq, dtype=np.float32)
    Wk = np.asarray(Wk, dtype=np.float32)
    Wv = np.asarray(Wv, dtype=np.float32)
    Wo = np.asarray(Wo, dtype=np.float32)
    scale = 1.0 / np.sqrt(DH)
    dt = ml_dtypes.bfloat16
    in_maps = []
    for c in range(N_CORES):
        b, hf = divmod(c, 2)
        cols = slice(hf * DQ, (hf + 1) * DQ)
        in_maps.append(
            {
                "xt": np.ascontiguousarray(x[b].T).astype(dt),
                "yt": np.ascontiguousarray(y[b].T).astype(dt),
                "wq": np.ascontiguousarray(Wq[:, cols] * scale).astype(dt),
                "wk": np.ascontiguousarray(Wk[:, cols]).astype(dt),
                "wv": np.ascontiguousarray(Wv[:, cols]).astype(dt),
                "wo": np.ascontiguousarray(Wo[cols, :]).astype(dt),
                "biasd": np.ascontiguousarray(bias[b, 0, 0].reshape(NK, P).T),
                "onesd": np.ones((P, NHL), dtype=dt),
            }
        )
    return in_maps


def get_program():
    global _CACHED_NC
    if _CACHED_NC is None:
        _CACHED_NC = build_program()
    return _CACHED_NC


def kernel(x, y, bias, Wq, Wk, Wv, Wo):
    nc = get_program()
    in_maps = _prep_in_maps(x, y, bias, Wq, Wk, Wv, Wo)
    res = bass_utils.run_bass_kernel_spmd(nc, in_maps, core_ids=list(range(N_CORES)))
    B = 4
    out = np.empty((B, S, HID), dtype=np.float32)
    for b in range(B):
        out[b] = res.results[2 * b]["out"].astype(np.float32) + res.results[
            2 * b + 1
        ]["out"].astype(np.float32)
    return out
